# revision 15
# baseline (speedup 1.0000x reference)
"""BevFeatureEncoder on 8 Trainium2 NeuronCores.

Strategy (data-parallel over BEV grid slabs):
  - The 2*480*360 BEV cells are split into 8 contiguous ranges of 43200
    cells (cores 0-3 own batch 0, x in [0,120)/[120,240)/...; cores 4-7
    own batch 1). Points are routed on host to the core owning their
    cell, so the segment_max reduction is fully local to each core.
  - On host (integer indexing only), each core's occupied cells are
    grouped by point count, counts padded up to k in {1,2,4,8,...} by
    duplicating points of the same cell (a no-op under max). Cells are
    processed in chunks of 512; points are laid out so slot s of a chunk
    is a dense, contiguous block of 512 points. The on-device
    segment_max is then dense tensor_max ops per chunk — no gather or
    scatter in the hot loop.
  - On device per 512-point slot block: 3-layer MLP as float32r matmuls
    (features on partitions, points on the free dim). BN scale is folded
    into the weights on device (diag(s) matmul + PE transposes at
    startup), so each BN+ReLU is a single per-partition bias+relu op,
    placeable on either the scalar or the vector engine. Max runs over
    raw layer-3 outputs (folded BN scale > 0, so bias+relu commute with
    max), bias+relu once per chunk, then the compression matmul
    (occupied cells only) and one contiguous DMA of the [64 ch, 512
    cell] result per chunk.
  - Device output is compacted chunk-major, channel-major [nchunks, 64,
    512]; the host unshard places real columns into the zeroed
    [B, C, GX, GY] grid (pure indexing, no float math).
  - Chunk structure is equalized across cores (padded with dummy cells,
    dropped at unshard) so a single SPMD program serves all 8 cores.
"""

import numpy as np

import concourse.bacc as bacc
import concourse.bass as bass
import concourse.mybir as mybir
import concourse.tile as tile
from concourse import bass_utils
from concourse.masks import make_identity

GX, GY = 480, 360
B = 2
EPS = 1e-5
N_CORES = 8
CELLS_PER_CORE = (B * GX * GY) // N_CORES  # 43200
CHUNK = 512  # cells per chunk
PAD = -1  # pad-cell marker in the row table

F32 = mybir.dt.float32
F32R = mybir.dt.float32r

Relu = mybir.ActivationFunctionType.Relu


# ---------------------------------------------------------------- host prep


def _prep_core(seg_local, lo_idx):
    """Group one core's occupied cells by padded point count.

    Returns {k: (cells [n], slots [n, k])}, point indices into the global
    voxel array, slots padded by repeating the last point of the cell.
    """
    order = np.argsort(seg_local, kind="stable")
    seg_sorted = seg_local[order]
    cells, starts, counts = np.unique(
        seg_sorted, return_index=True, return_counts=True
    )
    ks = 1 << (np.ceil(np.log2(np.maximum(counts, 1))).astype(np.int64))
    ks = np.maximum(ks, 1)
    out = {}
    for k in np.unique(ks):
        sel = np.nonzero(ks == k)[0]
        slots = np.empty((len(sel), int(k)), np.int64)
        for s in range(int(k)):
            slots[:, s] = order[starts[sel] + np.minimum(s, counts[sel] - 1)]
        out[int(k)] = (cells[sel].astype(np.int64), lo_idx[slots])
    return out


def _build_plan_and_data(voxels, coors):
    """Route points to cores, build the equalized chunk plan plus per-core
    device inputs (permuted voxels) and the host-side placement tables.

    chunk_plan: list of (k, c) with c a multiple of 128 (<= 512).
    """
    seg = (
        coors[:, 0].astype(np.int64) * (GX * GY)
        + coors[:, 1].astype(np.int64) * GY
        + coors[:, 2].astype(np.int64)
    )
    core_of = seg // CELLS_PER_CORE
    per_core = []
    for c in range(N_CORES):
        idx = np.nonzero(core_of == c)[0]
        per_core.append(_prep_core(seg[idx] - c * CELLS_PER_CORE, idx))

    all_ks = sorted({k for g in per_core for k in g.keys()})
    chunk_plan = []  # (k, c)
    for k in all_ks:
        n_max = max(len(g[k][0]) if k in g else 0 for g in per_core)
        n_pad = -(-n_max // 128) * 128
        while n_pad > 0:
            c = min(n_pad, CHUNK)
            if k > 1 and c == 384:
                # a [rel=c, c)-wide matmul write would cross a PSUM bank
                # boundary (wraps on HW); keep slot offsets bank-aligned
                chunk_plan.append((k, 256))
                n_pad -= 256
                continue
            chunk_plan.append((k, c))
            n_pad -= c
    total_pts = sum(k * c for k, c in chunk_plan)
    total_cells = sum(c for _, c in chunk_plan)

    vox_all = np.empty((N_CORES, 4, total_pts), np.float32)
    rows_all = np.full((N_CORES, total_cells), PAD, np.int64)

    for core in range(N_CORES):
        groups = per_core[core]
        col = 0
        cell0 = 0
        used = {}
        for k, c in chunk_plan:
            cells, slots = groups.get(
                k, (np.zeros(0, np.int64), np.zeros((0, k), np.int64)))
            u = used.get(k, 0)
            batch_cells = cells[u : u + c]
            batch_slots = slots[u : u + c]
            used[k] = u + c
            nb = len(batch_cells)
            sl = np.zeros((c, k), np.int64)
            if nb:
                sl[:nb] = batch_slots
                sl[nb:] = batch_slots[0, 0]
            elif len(cells):
                sl[:] = slots[0, 0]
            for s in range(k):
                vox_all[core, :, col : col + c] = voxels[sl[:, s]].T
                col += c
            rows_all[core, cell0 : cell0 + nb] = batch_cells
            cell0 += c
        assert col == total_pts and cell0 == total_cells
    return chunk_plan, total_pts, vox_all, rows_all


# ------------------------------------------------------------- bass program


def build_program(chunk_plan, total_pts):
    total_cells = sum(c for _, c in chunk_plan)
    nc = bacc.Bacc("TRN2", target_bir_lowering=False, debug=False,
                   num_devices=N_CORES)

    vox = nc.dram_tensor("vox", [4, total_pts], F32R,
                         kind="ExternalInput").ap()
    w_in = {}
    for name, shape in [
        ("w1", [4, 64]), ("w2", [64, 128]), ("w3", [128, 256]),
        ("wc", [256, 64]), ("bc", [64]),
        ("g1", [64]), ("b1", [64]), ("m1", [64]), ("v1", [64]),
        ("g2", [128]), ("b2", [128]), ("m2", [128]), ("v2", [128]),
        ("g3", [256]), ("b3", [256]), ("m3", [256]), ("v3", [256]),
    ]:
        w_in[name] = nc.dram_tensor(name, shape, F32, kind="ExternalInput").ap()
    comp = nc.dram_tensor("comp", [64, total_cells], F32,
                          kind="ExternalOutput").ap()

    from contextlib import ExitStack
    with tile.TileContext(nc) as tc, ExitStack() as ctx:
        cpool = ctx.enter_context(tc.tile_pool(name="const", bufs=1))

        ident = cpool.tile([128, 128], F32)
        make_identity(nc, ident[:])
        eps_tile = cpool.tile([128, 1], F32)
        nc.gpsimd.memset(eps_tile[:], EPS)

        def bn_vec(name, c, half=None):
            t = cpool.tile([c, 1], F32, tag=f"ld_{name}_{half}")
            src = w_in[name]
            if half is not None:
                src = src[half * 128 : half * 128 + c]
            nc.sync.dma_start(out=t[:], in_=src[:, None])
            return t

        def bn_scale_bias(li, c, half=None):
            sfx = f"{li}_{half}"
            g = bn_vec(f"g{li}", c, half)
            b = bn_vec(f"b{li}", c, half)
            m = bn_vec(f"m{li}", c, half)
            v = bn_vec(f"v{li}", c, half)
            sq = cpool.tile([c, 1], F32, tag=f"bn_sq{sfx}")
            nc.scalar.activation(sq[:], v[:], mybir.ActivationFunctionType.Sqrt,
                                 bias=eps_tile[:c, :])
            s = cpool.tile([c, 1], F32, tag=f"bn_s{sfx}")
            nc.vector.reciprocal(s[:], sq[:])
            nc.vector.tensor_mul(s[:], g[:], s[:])
            t = cpool.tile([c, 1], F32, tag=f"bn_t{sfx}")
            nc.vector.tensor_mul(t[:], m[:], s[:])
            nc.vector.tensor_sub(t[:], b[:], t[:])
            return s, t

        s1, t1 = bn_scale_bias(1, 64)
        s2, t2 = bn_scale_bias(2, 128)
        s3a, t3a = bn_scale_bias(3, 128, half=0)
        s3b, t3b = bn_scale_bias(3, 128, half=1)

        # ---- fold BN scale into weights: w' = w @ diag(s) ----
        with tc.tile_pool(name="fold_ps", bufs=1, space="PSUM") as fps, \
             tc.tile_pool(name="fold_sb", bufs=1) as fsb:

            def fold(name, kin, cout, w_src, s_ap):
                wr = fsb.tile([kin, cout], F32, tag="fold_raw")
                nc.sync.dma_start(out=wr[:], in_=w_src)
                pT = fps.tile([cout, kin], F32, tag="fold_pT", space="PSUM")
                nc.tensor.transpose(out=pT[:], in_=wr[:],
                                    identity=ident[:kin, :kin])
                wT = fsb.tile([cout, kin], F32, tag="fold_wT")
                nc.vector.tensor_copy(wT[:], pT[:])
                dg = fsb.tile([cout, cout], F32, tag="fold_dg")
                nc.vector.tensor_scalar_mul(dg[:], ident[:cout, :cout], s_ap)
                pS = fps.tile([cout, kin], F32, tag="fold_pS", space="PSUM")
                nc.tensor.matmul(pS[:], dg[:], wT[:], start=True, stop=True)
                wsT = fsb.tile([cout, kin], F32, tag="fold_wsT")
                nc.vector.tensor_copy(wsT[:], pS[:])
                pB = fps.tile([kin, cout], F32, tag="fold_pB", space="PSUM")
                nc.tensor.transpose(out=pB[:], in_=wsT[:],
                                    identity=ident[:cout, :cout])
                out = cpool.tile([kin, cout], F32R, tag=name)
                nc.vector.tensor_copy(out[:], pB[:])
                return out

            w1s = fold("w1s", 4, 64, w_in["w1"], s1[:])
            w2s = fold("w2s", 64, 128, w_in["w2"], s2[:])
            w3a = fold("w3a", 128, 128, w_in["w3"][:, 0:128], s3a[:])
            w3b = fold("w3b", 128, 128, w_in["w3"][:, 128:256], s3b[:])

        def load_round(name, shape, src_ap):
            raw = cpool.tile(shape, F32, tag=name + "_raw")
            nc.sync.dma_start(out=raw[:], in_=src_ap)
            rnd = cpool.tile(shape, F32R, tag=name)
            nc.vector.tensor_copy(rnd[:], raw[:])
            return rnd

        wc0 = load_round("wc0", [128, 64], w_in["wc"][0:128, :])
        wc1 = load_round("wc1", [128, 64], w_in["wc"][128:256, :])
        bc = cpool.tile([64, 1], F32)
        nc.sync.dma_start(out=bc[:], in_=w_in["bc"][:, None])

        sb = ctx.enter_context(tc.tile_pool(name="sb", bufs=2))
        vxp = ctx.enter_context(tc.tile_pool(name="vx", bufs=3))
        # PSUM budget (8 banks): p12 ring 1x[128,1024] (2 banks shared by
        # p1/p2), p3 2x[128,1024] (4), pc 1x[64,1024] (2)
        p12 = ctx.enter_context(tc.tile_pool(name="p12", bufs=1, space="PSUM"))
        ps3 = ctx.enter_context(tc.tile_pool(name="ps3", bufs=2, space="PSUM"))
        pcp = ctx.enter_context(tc.tile_pool(name="pcp", bufs=1, space="PSUM"))

        def br_act(out_ap, in_ap, bias_ap):
            """out = relu(in + bias) on the scalar engine, split to <=512
            free so a PSUM read never crosses a bank boundary."""
            W = in_ap.shape[-1]
            for o in range(0, W, 512):
                e = min(o + 512, W)
                nc.scalar.activation(out_ap[:, o:e], in_ap[:, o:e], Relu,
                                     bias=bias_ap, scale=1.0)

        def br_dve(out_ap, in_ap, bias_ap):
            """out = relu(in + bias) on the vector engine."""
            nc.vector.tensor_scalar(
                out_ap, in_ap, bias_ap, 0.0,
                op0=mybir.AluOpType.add, op1=mybir.AluOpType.max)

        def max_br_dve(out_ap, in_ap, bias_ap, acc_ap):
            """out = max(acc, in + bias); acc already relu'd (>= 0)."""
            nc.vector.scalar_tensor_tensor(
                out_ap, in_ap, bias_ap, acc_ap,
                op0=mybir.AluOpType.add, op1=mybir.AluOpType.max)

        def mlp_to_p3(segs, W):
            """segs: list of (vx_ap_slice_start, rel, c) describing point
            blocks mapped to columns [rel, rel+c) of the W-wide tiles.
            Returns (p3A, p3B) PSUM tiles [128, W]."""
            p1 = p12.tile([64, W], F32, tag="p12", space="PSUM")
            for vxap, rel, c in segs:
                nc.tensor.matmul(p1[:, rel : rel + c], w1s[:], vxap,
                                 start=True, stop=True)
            h1 = sb.tile([64, W], F32R, tag="h1")
            br_dve(h1[:], p1[:], t1[:])
            p2 = p12.tile([128, W], F32, tag="p12", space="PSUM")
            for _, rel, c in segs:
                nc.tensor.matmul(p2[:, rel : rel + c], w2s[:],
                                 h1[:, rel : rel + c], start=True, stop=True)
            h2 = sb.tile([128, W], F32R, tag="h2")
            br_act(h2[:], p2[:], t2[:])
            p3A = ps3.tile([128, W], F32, tag="p3", space="PSUM")
            p3B = ps3.tile([128, W], F32, tag="p3", space="PSUM")
            for _, rel, c in segs:
                nc.tensor.matmul(p3A[:, rel : rel + c], w3a[:],
                                 h2[:, rel : rel + c], start=True, stop=True)
            for _, rel, c in segs:
                nc.tensor.matmul(p3B[:, rel : rel + c], w3b[:],
                                 h2[:, rel : rel + c], start=True, stop=True)
            return p3A, p3B

        def compress(accrA_t, accrB_t, W, cell_off):
            """relu(max_feats @ wc + bc) -> comp[:, cell_off:cell_off+W]."""
            pc = pcp.tile([64, W], F32, tag="pc", space="PSUM")
            for o in range(0, W, 512):
                e = min(o + 512, W)
                nc.tensor.matmul(pc[:, o:e], wc0[:], accrA_t[:, o:e],
                                 start=True, stop=False)
            for o in range(0, W, 512):
                e = min(o + 512, W)
                nc.tensor.matmul(pc[:, o:e], wc1[:], accrB_t[:, o:e],
                                 start=False, stop=True)
            sc = sb.tile([64, W], F32, tag="sc")
            br_act(sc[:], pc[:], bc[:])
            nc.gpsimd.dma_start(out=comp[:, cell_off : cell_off + W], in_=sc[:])

        # ---- group chunks into units ----
        units = []  # ("k1", [(ci, c), (ci, c)?]) or ("kn", k, c, ci)
        pend = None
        pt_off = 0
        cell_off = 0
        meta = []  # per chunk: (k, c, pt_off, cell_off)
        for k, c in chunk_plan:
            meta.append((k, c, pt_off, cell_off))
            pt_off += k * c
            cell_off += c
        for i, (k, c, po, co) in enumerate(meta):
            if k == 1:
                if pend is None:
                    pend = i
                else:
                    units.append(("k1", [pend, i]))
                    pend = None
            else:
                units.append(("kn", i))
        if pend is not None:
            units.append(("k1", [pend]))

        for unit in units:
            if unit[0] == "k1":
                idxs = unit[1]
                W = sum(meta[i][1] for i in idxs)
                po0 = meta[idxs[0]][2]
                co0 = meta[idxs[0]][3]
                vx = vxp.tile([4, W], F32R, tag="vx")
                nc.sync.dma_start(out=vx[:], in_=vox[:, po0 : po0 + W])
                segs = []
                rel = 0
                for i in idxs:
                    c = meta[i][1]
                    segs.append((vx[:, rel : rel + c], rel, c))
                    rel += c
                p3A, p3B = mlp_to_p3(segs, W)
                accrA = sb.tile([128, W], F32R, tag="accrA")
                br_act(accrA[:], p3A[:], t3a[:])
                accrB = sb.tile([128, W], F32R, tag="accrB")
                br_dve(accrB[:], p3B[:], t3b[:])
                compress(accrA[:], accrB[:], W, co0)
            else:
                i = unit[1]
                k, c, po, co = meta[i]
                vx = vxp.tile([4, k * c], F32R, tag="vx")
                nc.sync.dma_start(out=vx[:], in_=vox[:, po : po + k * c])
                nA = nB = 0
                accA = [sb.tile([128, c], F32R, tag="accrA", name=f"accA{i}_{j}")
                        for j in range(2)]
                accB = [sb.tile([128, c], F32R, tag="accrB", name=f"accB{i}_{j}")
                        for j in range(2)]
                for s0 in range(0, k, 2):
                    W = 2 * c
                    segs = [(vx[:, s0 * c : s0 * c + c], 0, c),
                            (vx[:, (s0 + 1) * c : (s0 + 2) * c], c, c)]
                    p3A, p3B = mlp_to_p3(segs, W)
                    if s0 == 0:
                        br_act(accA[0][:], p3A[:, 0:c], t3a[:])
                        br_dve(accB[0][:], p3B[:, 0:c], t3b[:])
                    else:
                        max_br_dve(accA[1 - nA][:], p3A[:, 0:c], t3a[:],
                                   accA[nA][:]); nA = 1 - nA
                        max_br_dve(accB[1 - nB][:], p3B[:, 0:c], t3b[:],
                                   accB[nB][:]); nB = 1 - nB
                    max_br_dve(accA[1 - nA][:], p3A[:, c : 2 * c], t3a[:],
                               accA[nA][:]); nA = 1 - nA
                    max_br_dve(accB[1 - nB][:], p3B[:, c : 2 * c], t3b[:],
                               accB[nB][:]); nB = 1 - nB
                compress(accA[nA][:], accB[nB][:], c, co)

    nc.compile()
    return nc


# ------------------------------------------------------------------ driver

_CACHE = {}


def kernel(voxels, coors, batch_size, w1, g1, b1, m1, v1,
           w2, g2, b2, m2, v2, w3, g3, b3, m3, v3, wc, bc,
           _trace=False):
    voxels = np.asarray(voxels, np.float32)
    coors = np.asarray(coors, np.int32)
    chunk_plan, total_pts, vox_all, rows_all = _build_plan_and_data(
        voxels, coors)

    key = tuple(chunk_plan)
    if key not in _CACHE:
        _CACHE[key] = build_program(chunk_plan, total_pts)
    nc = _CACHE[key]

    weights = {
        k: np.asarray(v, np.float32)
        for k, v in [
            ("w1", w1), ("w2", w2), ("w3", w3), ("wc", wc), ("bc", bc),
            ("g1", g1), ("b1", b1), ("m1", m1), ("v1", v1),
            ("g2", g2), ("b2", b2), ("m2", m2), ("v2", v2),
            ("g3", g3), ("b3", b3), ("m3", m3), ("v3", v3),
        ]
    }
    in_maps = [{"vox": vox_all[c], **weights} for c in range(N_CORES)]
    res = bass_utils.run_bass_kernel_spmd(
        nc, in_maps, core_ids=list(range(N_CORES)), trace=_trace)

    # unshard: place compacted columns into the zeroed channel-major grid
    out = np.zeros((B, 64, GX * GY), np.float32)
    for c in range(N_CORES):
        cm = res.results[c]["comp"]  # [64, total_cells]
        rows = rows_all[c]  # [total_cells] local slab rows, PAD for dummy
        real = rows != PAD
        gcell = rows[real] + c * CELLS_PER_CORE
        b_core = c // (N_CORES // B)
        xy = gcell - b_core * (GX * GY)
        out[b_core][:, xy] = cm[:, real]
    out = out.reshape(B, 64, GX, GY)
    if _trace:
        return out, res
    return out


# revision 18
# speedup vs baseline: 1.1413x; 1.1413x over previous
"""BevFeatureEncoder on 8 Trainium2 NeuronCores.

Strategy (data-parallel over BEV grid slabs):
  - The 2*480*360 BEV cells are split into 8 contiguous ranges of 43200
    cells (cores 0-3 own batch 0, x in [0,120)/[120,240)/...; cores 4-7
    own batch 1). Points are routed on host to the core owning their
    cell, so the segment_max reduction is fully local to each core.
  - On host (integer indexing only), each core's occupied cells are
    grouped by point count, counts padded up to k in {1,2,4,8,...} by
    duplicating points of the same cell (a no-op under max). Cells are
    processed in chunks of 512; points are laid out so slot s of a chunk
    is a dense, contiguous block of 512 points. The on-device
    segment_max is then dense tensor_max ops per chunk — no gather or
    scatter in the hot loop.
  - On device per 512-point slot block: 3-layer MLP as float32r matmuls
    (features on partitions, points on the free dim). BN scale is folded
    into the weights on device (diag(s) matmul + PE transposes at
    startup), so each BN+ReLU is a single per-partition bias+relu op,
    placeable on either the scalar or the vector engine. Max runs over
    raw layer-3 outputs (folded BN scale > 0, so bias+relu commute with
    max), bias+relu once per chunk, then the compression matmul
    (occupied cells only) and one contiguous DMA of the [64 ch, 512
    cell] result per chunk.
  - Device output is compacted chunk-major, channel-major [nchunks, 64,
    512]; the host unshard places real columns into the zeroed
    [B, C, GX, GY] grid (pure indexing, no float math).
  - Chunk structure is equalized across cores (padded with dummy cells,
    dropped at unshard) so a single SPMD program serves all 8 cores.
"""

import numpy as np

import concourse.bacc as bacc
import concourse.bass as bass
import concourse.mybir as mybir
import concourse.tile as tile
from concourse import bass_utils
from concourse.masks import make_identity

GX, GY = 480, 360
B = 2
EPS = 1e-5
N_CORES = 8
CELLS_PER_CORE = (B * GX * GY) // N_CORES  # 43200
CHUNK = 512  # cells per chunk
PAD = -1  # pad-cell marker in the row table

F32 = mybir.dt.float32
F32R = mybir.dt.float32r

Relu = mybir.ActivationFunctionType.Relu


# ---------------------------------------------------------------- host prep


def _prep_core(seg_local, lo_idx):
    """Group one core's occupied cells by padded point count.

    Returns {k: (cells [n], slots [n, k])}, point indices into the global
    voxel array, slots padded by repeating the last point of the cell.
    """
    order = np.argsort(seg_local, kind="stable")
    seg_sorted = seg_local[order]
    cells, starts, counts = np.unique(
        seg_sorted, return_index=True, return_counts=True
    )
    ks = 1 << (np.ceil(np.log2(np.maximum(counts, 1))).astype(np.int64))
    ks = np.maximum(ks, 1)
    out = {}
    for k in np.unique(ks):
        sel = np.nonzero(ks == k)[0]
        slots = np.empty((len(sel), int(k)), np.int64)
        for s in range(int(k)):
            slots[:, s] = order[starts[sel] + np.minimum(s, counts[sel] - 1)]
        out[int(k)] = (cells[sel].astype(np.int64), lo_idx[slots])
    return out


def _build_plan_and_data(voxels, coors):
    """Route points to cores, build the equalized chunk plan plus per-core
    device inputs (permuted voxels) and the host-side placement tables.

    chunk_plan: list of (k, c) with c a multiple of 128 (<= 512).
    """
    seg = (
        coors[:, 0].astype(np.int64) * (GX * GY)
        + coors[:, 1].astype(np.int64) * GY
        + coors[:, 2].astype(np.int64)
    )
    core_of = seg // CELLS_PER_CORE
    per_core = []
    for c in range(N_CORES):
        idx = np.nonzero(core_of == c)[0]
        per_core.append(_prep_core(seg[idx] - c * CELLS_PER_CORE, idx))

    all_ks = sorted({k for g in per_core for k in g.keys()})
    chunk_plan = []  # (k, c)
    for k in all_ks:
        n_max = max(len(g[k][0]) if k in g else 0 for g in per_core)
        n_pad = -(-n_max // 128) * 128
        while n_pad > 0:
            c = min(n_pad, CHUNK)
            if c == 384:
                # a [rel=c, c)-wide matmul write would cross a PSUM bank
                # boundary (wraps on HW); keep slot offsets bank-aligned
                chunk_plan.append((k, 256))
                n_pad -= 256
                continue
            chunk_plan.append((k, c))
            n_pad -= c
    total_pts = sum(k * c for k, c in chunk_plan)
    total_cells = sum(c for _, c in chunk_plan)

    vox_all = np.empty((N_CORES, 4, total_pts), np.float32)
    rows_all = np.full((N_CORES, total_cells), PAD, np.int64)

    for core in range(N_CORES):
        groups = per_core[core]
        col = 0
        cell0 = 0
        used = {}
        for k, c in chunk_plan:
            cells, slots = groups.get(
                k, (np.zeros(0, np.int64), np.zeros((0, k), np.int64)))
            u = used.get(k, 0)
            batch_cells = cells[u : u + c]
            batch_slots = slots[u : u + c]
            used[k] = u + c
            nb = len(batch_cells)
            sl = np.zeros((c, k), np.int64)
            if nb:
                sl[:nb] = batch_slots
                sl[nb:] = batch_slots[0, 0]
            elif len(cells):
                sl[:] = slots[0, 0]
            for s in range(k):
                vox_all[core, :, col : col + c] = voxels[sl[:, s]].T
                col += c
            rows_all[core, cell0 : cell0 + nb] = batch_cells
            cell0 += c
        assert col == total_pts and cell0 == total_cells
    return chunk_plan, total_pts, vox_all, rows_all


# ------------------------------------------------------------- bass program


def build_program(chunk_plan, total_pts):
    total_cells = sum(c for _, c in chunk_plan)
    nc = bacc.Bacc("TRN2", target_bir_lowering=False, debug=False,
                   num_devices=N_CORES)

    vox = nc.dram_tensor("vox", [4, total_pts], F32R,
                         kind="ExternalInput").ap()
    w_in = {}
    for name, shape in [
        ("w1", [4, 64]), ("w2", [64, 128]), ("w3", [128, 256]),
        ("wc", [256, 64]), ("bc", [64]),
        ("g1", [64]), ("b1", [64]), ("m1", [64]), ("v1", [64]),
        ("g2", [128]), ("b2", [128]), ("m2", [128]), ("v2", [128]),
        ("g3", [256]), ("b3", [256]), ("m3", [256]), ("v3", [256]),
    ]:
        w_in[name] = nc.dram_tensor(name, shape, F32, kind="ExternalInput").ap()
    comp = nc.dram_tensor("comp", [64, total_cells], F32,
                          kind="ExternalOutput").ap()

    from contextlib import ExitStack
    with tile.TileContext(nc) as tc, ExitStack() as ctx:
        cpool = ctx.enter_context(tc.tile_pool(name="const", bufs=1))

        ident = cpool.tile([128, 128], F32)
        make_identity(nc, ident[:])
        eps_tile = cpool.tile([128, 1], F32)
        nc.gpsimd.memset(eps_tile[:], EPS)

        def bn_vec(name, c, half=None):
            t = cpool.tile([c, 1], F32, tag=f"ld_{name}_{half}")
            src = w_in[name]
            if half is not None:
                src = src[half * 128 : half * 128 + c]
            nc.sync.dma_start(out=t[:], in_=src[:, None])
            return t

        def bn_scale_bias(li, c, half=None):
            sfx = f"{li}_{half}"
            g = bn_vec(f"g{li}", c, half)
            b = bn_vec(f"b{li}", c, half)
            m = bn_vec(f"m{li}", c, half)
            v = bn_vec(f"v{li}", c, half)
            sq = cpool.tile([c, 1], F32, tag=f"bn_sq{sfx}")
            nc.scalar.activation(sq[:], v[:], mybir.ActivationFunctionType.Sqrt,
                                 bias=eps_tile[:c, :])
            s = cpool.tile([c, 1], F32, tag=f"bn_s{sfx}")
            nc.vector.reciprocal(s[:], sq[:])
            nc.vector.tensor_mul(s[:], g[:], s[:])
            t = cpool.tile([c, 1], F32, tag=f"bn_t{sfx}")
            nc.vector.tensor_mul(t[:], m[:], s[:])
            nc.vector.tensor_sub(t[:], b[:], t[:])
            return s, t

        s1, t1 = bn_scale_bias(1, 64)
        s2, t2 = bn_scale_bias(2, 128)
        s3a, t3a = bn_scale_bias(3, 128, half=0)
        s3b, t3b = bn_scale_bias(3, 128, half=1)

        # doubled per-partition bias for partition-packed [128, c] L1 tiles
        t1d = cpool.tile([128, 1], F32)
        nc.vector.tensor_copy(t1d[0:64, :], t1[:])
        nc.vector.tensor_copy(t1d[64:128, :], t1[:])

        # ---- fold BN scale into weights: w' = w @ diag(s) ----
        with tc.tile_pool(name="fold_ps", bufs=1, space="PSUM") as fps, \
             tc.tile_pool(name="fold_sb", bufs=1) as fsb:

            def fold(name, kin, cout, w_src, s_ap):
                wr = fsb.tile([kin, cout], F32, tag="fold_raw")
                nc.sync.dma_start(out=wr[:], in_=w_src)
                pT = fps.tile([cout, kin], F32, tag="fold_pT", space="PSUM")
                nc.tensor.transpose(out=pT[:], in_=wr[:],
                                    identity=ident[:kin, :kin])
                wT = fsb.tile([cout, kin], F32, tag="fold_wT")
                nc.vector.tensor_copy(wT[:], pT[:])
                dg = fsb.tile([cout, cout], F32, tag="fold_dg")
                nc.vector.tensor_scalar_mul(dg[:], ident[:cout, :cout], s_ap)
                pS = fps.tile([cout, kin], F32, tag="fold_pS", space="PSUM")
                nc.tensor.matmul(pS[:], dg[:], wT[:], start=True, stop=True)
                wsT = fsb.tile([cout, kin], F32, tag="fold_wsT")
                nc.vector.tensor_copy(wsT[:], pS[:])
                pB = fps.tile([kin, cout], F32, tag="fold_pB", space="PSUM")
                nc.tensor.transpose(out=pB[:], in_=wsT[:],
                                    identity=ident[:cout, :cout])
                out = cpool.tile([kin, cout], F32R, tag=name)
                nc.vector.tensor_copy(out[:], pB[:])
                return out

            w1s = fold("w1s", 4, 64, w_in["w1"], s1[:])
            w2s = fold("w2s", 64, 128, w_in["w2"], s2[:])
            w3a = fold("w3a", 128, 128, w_in["w3"][:, 0:128], s3a[:])
            w3b = fold("w3b", 128, 128, w_in["w3"][:, 128:256], s3b[:])

        def load_round(name, shape, src_ap):
            raw = cpool.tile(shape, F32, tag=name + "_raw")
            nc.sync.dma_start(out=raw[:], in_=src_ap)
            rnd = cpool.tile(shape, F32R, tag=name)
            nc.vector.tensor_copy(rnd[:], raw[:])
            return rnd

        # w2 duplicated into both partition halves so mm2 can consume
        # partition-packed h1 tiles (lhsT/rhs must share base partition)
        w2d = cpool.tile([128, 128], F32R)
        nc.vector.tensor_copy(w2d[0:64, :], w2s[:])
        nc.vector.tensor_copy(w2d[64:128, :], w2s[:])

        wc0 = load_round("wc0", [128, 64], w_in["wc"][0:128, :])
        wc1 = load_round("wc1", [128, 64], w_in["wc"][128:256, :])
        bc = cpool.tile([64, 1], F32)
        nc.sync.dma_start(out=bc[:], in_=w_in["bc"][:, None])

        sb = ctx.enter_context(tc.tile_pool(name="sb", bufs=4))
        scp = ctx.enter_context(tc.tile_pool(name="scp", bufs=2))
        vxp = ctx.enter_context(tc.tile_pool(name="vx", bufs=3))
        # PSUM (8 banks): p12 ring 2x[128,512] (2), p3 ring 4x[128,512] (4),
        # pc ring 2x[64,512] (2)
        p12 = ctx.enter_context(tc.tile_pool(name="p12", bufs=2, space="PSUM"))
        ps3 = ctx.enter_context(tc.tile_pool(name="ps3", bufs=4, space="PSUM"))
        pcp = ctx.enter_context(tc.tile_pool(name="pcp", bufs=2, space="PSUM"))

        def br_act(out_ap, in_ap, bias_ap):
            nc.scalar.activation(out_ap, in_ap, Relu, bias=bias_ap, scale=1.0)

        def br_dve(out_ap, in_ap, bias_ap):
            nc.vector.tensor_scalar(
                out_ap, in_ap, bias_ap, 0.0,
                op0=mybir.AluOpType.add, op1=mybir.AluOpType.max)

        def max_br_dve(out_ap, in_ap, bias_ap, acc_ap):
            nc.vector.scalar_tensor_tensor(
                out_ap, in_ap, bias_ap, acc_ap,
                op0=mybir.AluOpType.add, op1=mybir.AluOpType.max)

        # flat slot-item stream: (chunk_idx, k, c, slot, pt_off)
        items = []
        pt = 0
        cell_off = []
        co = 0
        for ci, (k, c) in enumerate(chunk_plan):
            cell_off.append(co)
            co += c
            for s in range(k):
                items.append((ci, k, c, s, pt))
                pt += c

        # batched vox loads: consecutive items share one DMA (<=2048 pts)
        vx_ap = {}
        batch = []
        bpts = 0

        def flush_vox():
            nonlocal batch, bpts
            if not batch:
                return
            p0 = batch[0][4]
            vx = vxp.tile([4, bpts], F32R, tag="vx", name=f"vx{p0}")
            nc.sync.dma_start(out=vx[:], in_=vox[:, p0 : p0 + bpts])
            for it in batch:
                rel = it[4] - p0
                vx_ap[it[0:4]] = vx[:, rel : rel + it[2]]
            batch = []
            bpts = 0

        for it in items:
            if bpts + it[2] > 2048:
                flush_vox()
            batch.append(it)
            bpts += it[2]
        flush_vox()

        # chunk state
        accA = {}
        accB = {}

        def finalize_chunk(ci, k, c):
            a_ap, b_ap = accA.pop(ci), accB.pop(ci)
            pc = pcp.tile([64, c], F32, tag="pc", space="PSUM",
                          name=f"pc{ci}")
            nc.tensor.matmul(pc[:], wc0[:], a_ap, start=True, stop=False)
            nc.tensor.matmul(pc[:], wc1[:], b_ap, start=False, stop=True)
            sc = scp.tile([64, c], F32, tag="sc", name=f"sc{ci}")
            br_act(sc[:], pc[:], bc[:])
            o = cell_off[ci]
            nc.gpsimd.dma_start(out=comp[:, o : o + c], in_=sc[:])

        # p1 packing state: two consecutive equal-width items share one
        # [128, c] PSUM tile (partitions 0:64 / 64:128)
        pend = None  # (item, p1_tile)

        def emit_l2up(it, h1_ap, w2_ap):
            """mm2 -> h2 -> mm3a/b -> max/affine for one slot item."""
            ci, k, c, s, po = it
            p2 = p12.tile([128, c], F32, tag="p12", space="PSUM",
                          name=f"p2_{ci}_{s}")
            nc.tensor.matmul(p2[:], w2_ap, h1_ap, start=True, stop=True)
            h2 = sb.tile([128, c], F32R, tag="h2", name=f"h2_{ci}_{s}")
            br_act(h2[:], p2[:], t2[:])
            p3A = ps3.tile([128, c], F32, tag="p3", space="PSUM",
                           name=f"p3A_{ci}_{s}")
            p3B = ps3.tile([128, c], F32, tag="p3", space="PSUM",
                           name=f"p3B_{ci}_{s}")
            nc.tensor.matmul(p3A[:], w3a[:], h2[:], start=True, stop=True)
            nc.tensor.matmul(p3B[:], w3b[:], h2[:], start=True, stop=True)
            if k == 1:
                aA = sb.tile([128, c], F32R, tag="accrA", name=f"aA_{ci}")
                br_act(aA[:], p3A[:], t3a[:])
                aB = sb.tile([128, c], F32R, tag="accrB", name=f"aB_{ci}")
                br_dve(aB[:], p3B[:], t3b[:])
                accA[ci], accB[ci] = aA[:], aB[:]
            elif s == 0:
                aA = [sb.tile([128, c], F32R, tag="accrA",
                              name=f"aA_{ci}_{j}") for j in range(2)]
                aB = [sb.tile([128, c], F32R, tag="accrB",
                              name=f"aB_{ci}_{j}") for j in range(2)]
                br_act(aA[0][:], p3A[:], t3a[:])
                br_dve(aB[0][:], p3B[:], t3b[:])
                accA[ci] = aA
                accB[ci] = aB
                accA[f"n{ci}"] = 0
                accB[f"n{ci}"] = 0
            else:
                nA = accA[f"n{ci}"]
                max_br_dve(accA[ci][1 - nA][:], p3A[:], t3a[:],
                           accA[ci][nA][:])
                accA[f"n{ci}"] = 1 - nA
                nB = accB[f"n{ci}"]
                max_br_dve(accB[ci][1 - nB][:], p3B[:], t3b[:],
                           accB[ci][nB][:])
                accB[f"n{ci}"] = 1 - nB
            if s == k - 1:
                if k > 1:
                    nA, nB = accA.pop(f"n{ci}"), accB.pop(f"n{ci}")
                    accA[ci] = accA[ci][nA][:]
                    accB[ci] = accB[ci][nB][:]
                finalize_chunk(ci, k, c)

        def flush_pend():
            nonlocal pend
            if pend is None:
                return
            it, p1 = pend
            h1 = sb.tile([64, it[2]], F32R, tag="h1", name=f"h1s_{it[0]}_{it[3]}")
            br_dve(h1[:], p1[0:64, :], t1[:])
            emit_l2up(it, h1[:], w2d[0:64, :])
            pend = None

        for it in items:
            ci, k, c, s, po = it
            if False and pend is not None and pend[0][2] == c:
                it0, p1 = pend
                pend = None
                nc.tensor.matmul(p1[64:128, :], w1s[:], vx_ap[it[0:4]],
                                 start=True, stop=True)
                h1 = sb.tile([128, c], F32R, tag="h1",
                             name=f"h1_{ci}_{s}")
                br_dve(h1[:], p1[:], t1d[:])
                emit_l2up(it0, h1[0:64, :], w2d[0:64, :])
                emit_l2up(it, h1[64:128, :], w2d[64:128, :])
            else:
                flush_pend()
                p1 = p12.tile([128, c], F32, tag="p12", space="PSUM",
                              name=f"p1_{ci}_{s}")
                nc.tensor.matmul(p1[0:64, :], w1s[:], vx_ap[it[0:4]],
                                 start=True, stop=True)
                pend = (it, p1)
        flush_pend()

    nc.compile()
    return nc


# ------------------------------------------------------------------ driver

_CACHE = {}


def kernel(voxels, coors, batch_size, w1, g1, b1, m1, v1,
           w2, g2, b2, m2, v2, w3, g3, b3, m3, v3, wc, bc,
           _trace=False):
    voxels = np.asarray(voxels, np.float32)
    coors = np.asarray(coors, np.int32)
    chunk_plan, total_pts, vox_all, rows_all = _build_plan_and_data(
        voxels, coors)

    key = tuple(chunk_plan)
    if key not in _CACHE:
        _CACHE[key] = build_program(chunk_plan, total_pts)
    nc = _CACHE[key]

    weights = {
        k: np.asarray(v, np.float32)
        for k, v in [
            ("w1", w1), ("w2", w2), ("w3", w3), ("wc", wc), ("bc", bc),
            ("g1", g1), ("b1", b1), ("m1", m1), ("v1", v1),
            ("g2", g2), ("b2", b2), ("m2", m2), ("v2", v2),
            ("g3", g3), ("b3", b3), ("m3", m3), ("v3", v3),
        ]
    }
    in_maps = [{"vox": vox_all[c], **weights} for c in range(N_CORES)]
    res = bass_utils.run_bass_kernel_spmd(
        nc, in_maps, core_ids=list(range(N_CORES)), trace=_trace)

    # unshard: place compacted columns into the zeroed channel-major grid
    out = np.zeros((B, 64, GX * GY), np.float32)
    for c in range(N_CORES):
        cm = res.results[c]["comp"]  # [64, total_cells]
        rows = rows_all[c]  # [total_cells] local slab rows, PAD for dummy
        real = rows != PAD
        gcell = rows[real] + c * CELLS_PER_CORE
        b_core = c // (N_CORES // B)
        xy = gcell - b_core * (GX * GY)
        out[b_core][:, xy] = cm[:, real]
    out = out.reshape(B, 64, GX, GY)
    if _trace:
        return out, res
    return out
